# revision 11
# baseline (speedup 1.0000x reference)
"""Causal self-attention with KV cache — Trainium2 Bass kernel.

Strategy: tensor-parallel over heads. 16 heads / 8 cores = 2 heads per core.
Each core computes qkv projection for its 2 heads, causal attention, and a
partial output projection (its 128 columns of the c_proj contraction). The
host sums the 8 partial projections and assembles the k/v cache outputs.

Device layout is "transposed": qT/kT/vT live as [head*dk, tokens] so that
 - qkv projection runs with W stationary / xT moving (full PE efficiency),
 - score blocks come out as PT = scores.T [k, q] (softmax denom via a
   ones-column in the AV stationary operand),
 - attention output yT [c_local, tokens] is directly the lhsT the c_proj
   matmul needs (K=128, full efficiency).
All matmuls run as float32r (full rate at moving dim >= 256).
"""

import numpy as np

import concourse.bass as bass
import concourse.mybir as mybir
import concourse.tile as tile
from concourse import bacc
from concourse.bass_utils import run_bass_kernel_spmd
from concourse.masks import make_identity

F32 = mybir.dt.float32
F32R = mybir.dt.float32r

B = 2
T = 2048
C = 1024
N_HEADS = 16
DK = 64
N_CORES = 8
HL = N_HEADS // N_CORES      # heads per core = 2
CL = HL * DK                 # local channels = 128
QKVW = 3 * CL                # local qkv width = 384
KCH = C // 128               # emb contraction chunks = 8
QTILE = 512                  # q tile (moving dim)
NQT = T // QTILE             # q tiles per (b,h) = 4
NKC = T // 128               # k chunks per batch = 16
SCALE = 0.125                # 1/sqrt(DK)


def _r(ap):
    return ap.bitcast(F32R)


def _emit(nc, tc, io):
    xT_d, wqkv_d, bqkv_d, wproj_d, ypart_d, kout_d, vout_d = io

    pools = {}

    def pool(name, bufs, space="SBUF"):
        if name not in pools:
            pools[name] = tc.alloc_tile_pool(name=name, bufs=bufs, space=space)
        return pools[name]

    const_p = pool("const", 1)
    x_p = pool("x", 2)
    qkvT_p = pool("qkvT", 2)
    vn_p = pool("vn", 2)
    kn_p = pool("kn", 4)
    pt_p = pool("pt", 4)
    yt_p = pool("yt", 3)
    rec_p = pool("rec", 3)
    yo_p = pool("yo", 4)
    mm_ps = pool("mm_ps", 2, space="PSUM")
    pt_ps = pool("pt_ps", 3, space="PSUM")
    acc_ps = pool("acc_ps", 3, space="PSUM")

    # ---- constants ----
    w_sb = const_p.tile([128, KCH, QKVW], F32R, name="w_sb")
    for k in range(KCH):
        nc.sync.dma_start(w_sb[:, k, :], wqkv_d[k * 128:(k + 1) * 128, :])
    wp_sb = const_p.tile([128, C], F32R, name="wp_sb")
    nc.sync.dma_start(wp_sb[:], wproj_d[:, :])
    b_sb = const_p.tile([128, 3], F32, name="b_sb")
    for m in range(3):
        nc.sync.dma_start(b_sb[:, m:m + 1], bqkv_d[m:m + 1, :].rearrange("a p -> p a"))
    identity_f32 = const_p.tile([128, 128], F32, name="identity_f32")
    make_identity(nc, identity_f32)
    identity = const_p.tile([128, 128], F32R, name="identity")
    nc.vector.tensor_copy(identity[:], identity_f32[:])
    ones_f32 = const_p.tile([128, NKC * HL], F32, name="ones_f32")
    nc.gpsimd.memset(ones_f32[:], 1.0)
    # sel_h[0, m] = 1.0 iff m in head h's channel block (recip broadcast)
    sel = []
    for h in range(HL):
        sh = const_p.tile([1, 128], F32, name=f"sel{h}")
        nc.gpsimd.memset(sh[:], 0.0)
        nc.gpsimd.memset(sh[:, h * DK:(h + 1) * DK], 1.0)
        sel.append(sh)

    for b in range(B):
        # ---- qkv projection for batch b: qT/kT/vT [128, T] ----
        qT = qkvT_p.tile([128, T], F32R, name="qT", tag="qT")
        kT = qkvT_p.tile([128, T], F32R, name="kT", tag="kT")
        vT = qkvT_p.tile([128, T], F32R, name="vT", tag="vT")
        dests = (qT, kT, vT)
        for j in range(NQT):
            tt = b * NQT + j
            x_sb = x_p.tile([128, KCH, QTILE], F32R, name="x_sb")
            for k in range(KCH):
                nc.sync.dma_start(
                    x_sb[:, k, :],
                    xT_d[k * 128:(k + 1) * 128, tt * QTILE:(tt + 1) * QTILE])
            for m in range(3):
                ps = mm_ps.tile([128, QTILE], F32, name="qkv_ps", tag="mm")
                for k in range(KCH):
                    nc.tensor.matmul(
                        ps[:], w_sb[:, k, m * 128:(m + 1) * 128],
                        x_sb[:, k, :],
                        start=(k == 0), stop=(k == KCH - 1))
                nc.vector.tensor_scalar_add(
                    dests[m][:, j * QTILE:(j + 1) * QTILE], ps[:], b_sb[:, m:m + 1])

        # ---- transpose v (for AV stationary + v output), k (for k output) ----
        vn = vn_p.tile([128, NKC, HL, DK + 1], F32R, name="vn")
        nc.vector.tensor_copy(
            vn[:, :, :, DK:DK + 1],
            ones_f32.rearrange("p (a b c) -> p a b c", b=HL, c=1))
        for ci in range(NKC):
            trp = acc_ps.tile([128, 128], F32R, name="trp", tag="acc")
            nc.tensor.transpose(trp[:], vT[:, ci * 128:(ci + 1) * 128], identity[:])
            nc.vector.tensor_copy(
                vn[:, ci, :, 0:DK], trp.rearrange("p (h d) -> p h d", h=HL))
            nc.sync.dma_start(
                vout_d[b, ci * 128:(ci + 1) * 128, :], vn[:, ci, :, 0:DK].bitcast(F32))
            trk = acc_ps.tile([128, 128], F32R, name="trk", tag="acc")
            nc.tensor.transpose(trk[:], kT[:, ci * 128:(ci + 1) * 128], identity[:])
            kn = kn_p.tile([128, 128], F32, name="kn")
            nc.vector.tensor_copy(kn[:], trk[:])
            nc.sync.dma_start(kout_d[b, ci * 128:(ci + 1) * 128, :], kn[:])

        # ---- causal attention + partial c_proj, per q tile ----
        for qb in range(NQT):
            nkc = 4 * qb + 4
            rec = [rec_p.tile([1, QTILE], F32, name=f"rec{h}", tag=f"rec{h}")
                   for h in range(HL)]
            yt = yt_p.tile([128, QTILE], F32R, name="yt")
            for h in range(HL):
                kTh = kT[h * DK:(h + 1) * DK, :]
                qTh = qT[h * DK:(h + 1) * DK, qb * QTILE:(qb + 1) * QTILE]
                y_ps = acc_ps.tile([DK + 1, QTILE], F32, name="y_ps", tag="acc")
                for kc in range(nkc):
                    p_ps = pt_ps.tile([128, QTILE], F32, name="p_ps", tag="pt")
                    nc.tensor.matmul(
                        p_ps[:], kTh[:, kc * 128:(kc + 1) * 128], qTh,
                        start=True, stop=True)
                    p_sb = pt_p.tile([128, QTILE], F32R, name="p_sb")
                    nc.scalar.activation(
                        p_sb[:], p_ps[:], mybir.ActivationFunctionType.Exp,
                        scale=SCALE)
                    r = kc - 4 * qb
                    if r >= 0:
                        # keep where kp + 128*r <= qf, else 0
                        nc.gpsimd.affine_select(
                            out=p_sb[:], in_=p_sb[:],
                            compare_op=mybir.AluOpType.is_ge, fill=0.0,
                            base=-128 * r, channel_multiplier=-1,
                            pattern=[[1, QTILE]])
                    nc.tensor.matmul(
                        y_ps[:], vn[:, kc, h, :], p_sb[:],
                        start=(kc == 0), stop=(kc == nkc - 1))
                nc.vector.reciprocal(rec[h][:], y_ps[DK:DK + 1, :])
                nc.vector.tensor_copy(yt[h * DK:(h + 1) * DK, :], y_ps[0:DK, :])
            # broadcast per-head reciprocals across partitions: [2,q] -> [128,q]
            br_ps = mm_ps.tile([128, QTILE], F32, name="br_ps", tag="mm")
            for h in range(HL):
                nc.tensor.matmul(br_ps[:], sel[h][:], rec[h][:],
                                 start=(h == 0), stop=(h == HL - 1))
            nc.vector.tensor_mul(yt[:], yt[:], br_ps[:])
            # partial projection: y_part[tok, :] += yt.T @ wp  (K = 128)
            for ts in range(4):
                r0 = b * T + qb * QTILE + ts * 128
                for nh in range(2):
                    pp = mm_ps.tile([128, 512], F32, name="pp", tag="mm")
                    nc.tensor.matmul(
                        pp[:], yt[:, ts * 128:(ts + 1) * 128],
                        wp_sb[:, nh * 512:(nh + 1) * 512],
                        start=True, stop=True)
                    yo = yo_p.tile([128, 512], F32, name="yo")
                    nc.vector.tensor_copy(yo[:], pp[:])
                    nc.sync.dma_start(
                        ypart_d[r0:r0 + 128, nh * 512:(nh + 1) * 512], yo[:])

    for p in reversed(list(pools.values())):
        p.release()


def build_program():
    nc = bacc.Bacc("TRN2", target_bir_lowering=False, debug=False,
                   enable_asserts=False, num_devices=N_CORES)
    xT_d = nc.dram_tensor("xT", [C, B * T], F32R, kind="ExternalInput").ap()
    wqkv_d = nc.dram_tensor("wqkv", [C, QKVW], F32R, kind="ExternalInput").ap()
    bqkv_d = nc.dram_tensor("bqkv", [3, CL], F32, kind="ExternalInput").ap()
    wproj_d = nc.dram_tensor("wproj", [CL, C], F32R, kind="ExternalInput").ap()
    ypart_d = nc.dram_tensor("y_part", [B * T, C], F32, kind="ExternalOutput").ap()
    kout_d = nc.dram_tensor("k_out", [B, T, CL], F32, kind="ExternalOutput").ap()
    vout_d = nc.dram_tensor("v_out", [B, T, CL], F32, kind="ExternalOutput").ap()
    io = (xT_d, wqkv_d, bqkv_d, wproj_d, ypart_d, kout_d, vout_d)
    with tile.TileContext(nc) as tc:
        _emit(nc, tc, io)
    nc.compile()
    return nc


def shard_inputs(x, W_qkv, b_qkv, W_proj):
    x = np.asarray(x, np.float32)
    W_qkv = np.asarray(W_qkv, np.float32)
    b_qkv = np.asarray(b_qkv, np.float32)
    W_proj = np.asarray(W_proj, np.float32)
    xT = np.ascontiguousarray(x.reshape(B * T, C).T)
    in_maps = []
    for c in range(N_CORES):
        s = c * CL
        w = np.ascontiguousarray(np.concatenate(
            [W_qkv[:, s:s + CL], W_qkv[:, C + s:C + s + CL],
             W_qkv[:, 2 * C + s:2 * C + s + CL]], axis=1))
        bq = np.ascontiguousarray(np.stack(
            [b_qkv[s:s + CL], b_qkv[C + s:C + s + CL],
             b_qkv[2 * C + s:2 * C + s + CL]]).reshape(3, CL))
        wp = np.ascontiguousarray(W_proj[s:s + CL, :])
        in_maps.append({"xT": xT, "wqkv": w, "bqkv": bq, "wproj": wp})
    return in_maps


def gather_outputs(results, b_proj):
    b_proj = np.asarray(b_proj, np.float32)
    y = np.zeros((B * T, C), np.float64)
    for r in results:
        y += r["y_part"]
    y = (y + b_proj).astype(np.float32).reshape(B, T, C)
    k = np.concatenate(
        [r["k_out"].reshape(B, T, HL, DK) for r in results], axis=2)
    v = np.concatenate(
        [r["v_out"].reshape(B, T, HL, DK) for r in results], axis=2)
    k = np.ascontiguousarray(k.transpose(0, 2, 1, 3))
    v = np.ascontiguousarray(v.transpose(0, 2, 1, 3))
    return y, k, v


_NC = None


def _get_nc():
    global _NC
    if _NC is None:
        _NC = build_program()
    return _NC


def kernel(x, W_qkv, b_qkv, W_proj, b_proj, **run_kwargs):
    nc = _get_nc()
    in_maps = shard_inputs(x, W_qkv, b_qkv, W_proj)
    res = run_bass_kernel_spmd(nc, in_maps, core_ids=list(range(N_CORES)),
                               **run_kwargs)
    y, k, v = gather_outputs(res.results, b_proj)
    kernel.last_results = res
    return (y, (k, v))


# revision 33
# speedup vs baseline: 1.1562x; 1.1562x over previous
"""Causal self-attention with KV cache — Trainium2 Bass kernel.

Strategy: tensor-parallel over heads. 16 heads / 8 cores = 2 heads per core.
Each core computes qkv projection for its 2 heads, causal attention, and a
partial output projection (its 128 columns of the c_proj contraction). The
host sums the 8 partial projections and assembles the k/v cache outputs.

Device layout is "transposed": qT/kT/vT live as [head*dk, tokens] so that
 - qkv projection runs with W stationary / xT moving (full PE efficiency),
 - score blocks come out as PT = scores.T [k, q] (softmax denom via a
   ones-column in the AV stationary operand),
 - attention output yT [c_local, tokens] is directly the lhsT the c_proj
   matmul needs (K=128, full efficiency).
All matmuls run as float32r (full rate at moving dim >= 256).
"""

import numpy as np

import concourse.bass as bass
import concourse.mybir as mybir
import concourse.tile as tile
from concourse import bacc
from concourse.bass_utils import run_bass_kernel_spmd
from concourse.masks import make_identity

F32 = mybir.dt.float32
F32R = mybir.dt.float32r

B = 2
T = 2048
C = 1024
N_HEADS = 16
DK = 64
N_CORES = 8
HL = N_HEADS // N_CORES      # heads per core = 2
CL = HL * DK                 # local channels = 128
QKVW = 3 * CL                # local qkv width = 384
KCH = C // 128               # emb contraction chunks = 8
QTILE = 512                  # q tile (moving dim)
NQT = T // QTILE             # q tiles per (b,h) = 4
NKC = T // 128               # k chunks per batch = 16
SCALE = 0.125                # 1/sqrt(DK)


def _r(ap):
    return ap.bitcast(F32R)


def _emit(nc, tc, io):
    xT_d, wqkv_d, bqkv_d, wproj_d, ypart_d, kout_d, vout_d = io

    pools = {}

    def pool(name, bufs, space="SBUF"):
        if name not in pools:
            pools[name] = tc.alloc_tile_pool(name=name, bufs=bufs, space=space)
        return pools[name]

    const_p = pool("const", 1)
    x_p = pool("x", 2)
    qkvT_p = pool("qkvT", 2)
    vn_p = pool("vn", 2)
    kn_p = pool("kn", 2)
    pt_p = pool("pt", 3)
    yt_p = pool("yt", 3)
    rec_p = pool("rec", 2)
    yo_p = pool("yo", 2)
    mm_ps = pool("mm_ps", 2, space="PSUM")
    pt_ps = pool("pt_ps", 4, space="PSUM")
    acc_ps = pool("acc_ps", 2, space="PSUM")

    # ---- constants ----
    w_sb = const_p.tile([128, KCH, QKVW], F32R, name="w_sb")
    nc.sync.dma_start(w_sb[:], wqkv_d.rearrange("(k p) n -> p k n", p=128))
    wp_sb = const_p.tile([128, C], F32R, name="wp_sb")
    nc.sync.dma_start(wp_sb[:], wproj_d[:, :])
    b_sb = const_p.tile([128, 3], F32, name="b_sb")
    nc.sync.dma_start(b_sb[:], bqkv_d.rearrange("m p -> p m"))
    identity_f32 = const_p.tile([128, 128], F32, name="identity_f32")
    make_identity(nc, identity_f32)
    identity = const_p.tile([128, 128], F32R, name="identity")
    nc.vector.tensor_copy(identity[:], identity_f32[:])
    ones_f32 = const_p.tile([128, NKC * HL], F32, name="ones_f32")
    nc.gpsimd.memset(ones_f32[:], 1.0)
    # sel_h[0, m] = 1.0 iff m in head h's channel block (recip broadcast)
    sel = []
    for h in range(HL):
        sf = const_p.tile([1, 128], F32, name=f"self{h}")
        nc.gpsimd.memset(sf[:], 0.0)
        nc.gpsimd.memset(sf[:, h * DK:(h + 1) * DK], 1.0)
        sh = const_p.tile([1, 128], F32R, name=f"sel{h}")
        nc.vector.tensor_copy(sh[:], sf[:])
        sel.append(sh)

    def emit_qkv(b, j, dests):
        qT, kT, vT = dests
        tt = b * NQT + j
        x_sb = x_p.tile([128, KCH, QTILE], F32R, name="x_sb")
        for k in range(KCH):
            nc.sync.dma_start(
                x_sb[:, k, :],
                xT_d[k * 128:(k + 1) * 128, tt * QTILE:(tt + 1) * QTILE])
        yield
        for m in range(3):
            ps = mm_ps.tile([128, QTILE], F32, name="qkv_ps", tag="mm")
            for k in range(KCH):
                nc.tensor.matmul(
                    ps[:], w_sb[:, k, m * 128:(m + 1) * 128], x_sb[:, k, :],
                    start=(k == 0), stop=(k == KCH - 1))
            nc.vector.tensor_scalar_add(
                (qT, kT, vT)[m][:, j * QTILE:(j + 1) * QTILE], ps[:],
                b_sb[:, m:m + 1])
            yield

    def emit_transposes_j(dests, vn, kn, j):
        qT, kT, vT = dests
        for ci in range(4 * j, 4 * j + 4):
            trp = acc_ps.tile([128, 128], F32R, name="trp", tag="acc")
            nc.tensor.transpose(
                trp[:], vT[:, ci * 128:(ci + 1) * 128], identity[:])
            nc.scalar.copy(
                vn[:, ci, :, 0:DK], trp.rearrange("p (h d) -> p h d", h=HL))
            trk = acc_ps.tile([128, 128], F32R, name="trk", tag="acc")
            nc.tensor.transpose(
                trk[:], kT[:, ci * 128:(ci + 1) * 128], identity[:])
            nc.scalar.copy(kn[:, ci, :], trk[:])

    def emit_attention(b, qb, dests, vn):
        """One q-tile: both heads' chains interleaved pair-by-pair."""
        qT, kT, vT = dests
        npair = 2 * qb + 2             # k-chunk pairs (chunks 0..4qb+3)
        rec = [rec_p.tile([1, QTILE], F32R, name=f"rec{h}", tag=f"rec{h}")
               for h in range(HL)]
        yt = yt_p.tile([128, QTILE], F32R, name="yt")
        y_ps = [acc_ps.tile([DK + 1, QTILE], F32, name=f"y_ps{h}", tag="acc")
                for h in range(HL)]
        for kc in range(4 * qb + 4):
            r = kc - 4 * qb
            # chunk kc only reaches q columns >= 128*r (causal)
            w0 = max(0, 128 * r)
            p_sbs = []
            for h in range(HL):
                kTh = kT[h * DK:(h + 1) * DK, :]
                qTh = qT[h * DK:(h + 1) * DK, qb * QTILE:(qb + 1) * QTILE]
                p_ps = pt_ps.tile([128, QTILE], F32, name="p_ps", tag="pt")
                nc.tensor.matmul(
                    p_ps[:, w0:], kTh[:, kc * 128:(kc + 1) * 128],
                    qTh[:, w0:], start=True, stop=True)
                p_sb = pt_p.tile([128, QTILE], F32R, name=f"p_sb{h}",
                                 tag=f"p_sb{h}")
                nc.scalar.activation(
                    p_sb[:, w0:], p_ps[:, w0:],
                    mybir.ActivationFunctionType.Exp, scale=SCALE)
                p_sbs.append(p_sb)
            for h in range(HL):
                if r >= 0:
                    # keep where kp <= qf - 128*r (triangle at window start)
                    nc.gpsimd.affine_select(
                        out=p_sbs[h][:, w0:], in_=p_sbs[h][:, w0:],
                        compare_op=mybir.AluOpType.is_ge, fill=0.0,
                        base=0, channel_multiplier=-1,
                        pattern=[[1, QTILE - w0]])
                nc.tensor.matmul(
                    y_ps[h][:, w0:], vn[:, kc, h, :], p_sbs[h][:, w0:],
                    start=(kc == 0), stop=(kc == 4 * qb + 3))
            yield
        for h in range(HL):
            with nc.allow_low_precision(reason="fp32r softmax recip"):
                nc.vector.reciprocal(rec[h][:], y_ps[h][DK:DK + 1, :])
            nc.vector.tensor_copy(yt[h * DK:(h + 1) * DK, :], y_ps[h][0:DK, :])
        # broadcast per-head reciprocals across partitions: [1,q] -> [128,q]
        br_ps = mm_ps.tile([128, QTILE], F32, name="br_ps", tag="mm")
        for h in range(HL):
            nc.tensor.matmul(br_ps[:], sel[h][:], rec[h][:],
                             start=(h == 0), stop=(h == HL - 1))
        nc.vector.tensor_mul(yt[:], yt[:], br_ps[:])
        # partial projection: y_part[tok, :] += yt.T @ wp  (K = 128)
        for ts in range(4):
            r0 = b * T + qb * QTILE + ts * 128
            yo = yo_p.tile([128, 2, 512], F32, name="yo")
            for nh in range(2):
                pp = mm_ps.tile([128, 512], F32, name="pp", tag="mm")
                nc.tensor.matmul(
                    pp[:], yt[:, ts * 128:(ts + 1) * 128],
                    wp_sb[:, nh * 512:(nh + 1) * 512],
                    start=True, stop=True)
                nc.vector.tensor_copy(yo[:, nh, :], pp[:])
            nc.sync.dma_start(
                ypart_d[r0:r0 + 128, :], yo.rearrange("p a b -> p (a b)"))
            yield

    def emit_kv_out(b, vn, kn):
        for h in range(HL):
            nc.sync.dma_start(
                vout_d[b].rearrange("(c p) (h d) -> p c h d", p=128, h=HL)[
                    :, :, h, :],
                vn[:, :, h, 0:DK].bitcast(F32))
        nc.sync.dma_start(
            kout_d[b].rearrange("(c p) n -> p c n", p=128), kn[:])

    dests, vns, kns = [], [], []
    for b in range(B):
        qT = qkvT_p.tile([128, T], F32R, name=f"qT{b}", tag="qT")
        kT = qkvT_p.tile([128, T], F32R, name=f"kT{b}", tag="kT")
        vT = qkvT_p.tile([128, T], F32R, name=f"vT{b}", tag="vT")
        dests.append((qT, kT, vT))
        vns.append(vn_p.tile([128, NKC, HL, DK + 1], F32R, name=f"vn{b}"))
        kns.append(kn_p.tile([128, NKC, 128], F32, name=f"kn{b}"))

    for b in range(B):
        nc.vector.tensor_copy(
            vns[b][:, :, :, DK:DK + 1],
            ones_f32.rearrange("p (a b c) -> p a b c", b=HL, c=1))

    def drive(*gens):
        live = [g for g in gens if g is not None]
        while live:
            for g in list(live):
                try:
                    next(g)
                except StopIteration:
                    live.remove(g)

    # (batch, qtile) pairs in order; attention(t) interleaved with qkv(t+1)
    tiles = [(0, j) for j in range(NQT)] + [(1, j) for j in range(NQT)]
    drive(emit_qkv(0, 0, dests[0]))
    for idx, (b, qb) in enumerate(tiles):
        emit_transposes_j(dests[b], vns[b], kns[b], qb)
        nxt = tiles[idx + 1] if idx + 1 < len(tiles) else None
        qkv_gen = (emit_qkv(nxt[0], nxt[1], dests[nxt[0]])
                   if nxt is not None else None)
        drive(emit_attention(b, qb, dests[b], vns[b]), qkv_gen)
        if qb == NQT - 1:
            emit_kv_out(b, vns[b], kns[b])

    for p in reversed(list(pools.values())):
        p.release()


def build_program():
    nc = bacc.Bacc("TRN2", target_bir_lowering=False, debug=False,
                   enable_asserts=False, num_devices=N_CORES)
    xT_d = nc.dram_tensor("xT", [C, B * T], F32R, kind="ExternalInput").ap()
    wqkv_d = nc.dram_tensor("wqkv", [C, QKVW], F32R, kind="ExternalInput").ap()
    bqkv_d = nc.dram_tensor("bqkv", [3, CL], F32, kind="ExternalInput").ap()
    wproj_d = nc.dram_tensor("wproj", [CL, C], F32R, kind="ExternalInput").ap()
    ypart_d = nc.dram_tensor("y_part", [B * T, C], F32, kind="ExternalOutput").ap()
    kout_d = nc.dram_tensor("k_out", [B, T, CL], F32, kind="ExternalOutput").ap()
    vout_d = nc.dram_tensor("v_out", [B, T, CL], F32, kind="ExternalOutput").ap()
    io = (xT_d, wqkv_d, bqkv_d, wproj_d, ypart_d, kout_d, vout_d)
    with tile.TileContext(nc) as tc:
        _emit(nc, tc, io)
    nc.compile()
    return nc


def shard_inputs(x, W_qkv, b_qkv, W_proj):
    x = np.asarray(x, np.float32)
    W_qkv = np.asarray(W_qkv, np.float32)
    b_qkv = np.asarray(b_qkv, np.float32)
    W_proj = np.asarray(W_proj, np.float32)
    xT = np.ascontiguousarray(x.reshape(B * T, C).T)
    in_maps = []
    for c in range(N_CORES):
        s = c * CL
        w = np.ascontiguousarray(np.concatenate(
            [W_qkv[:, s:s + CL], W_qkv[:, C + s:C + s + CL],
             W_qkv[:, 2 * C + s:2 * C + s + CL]], axis=1))
        bq = np.ascontiguousarray(np.stack(
            [b_qkv[s:s + CL], b_qkv[C + s:C + s + CL],
             b_qkv[2 * C + s:2 * C + s + CL]]).reshape(3, CL))
        wp = np.ascontiguousarray(W_proj[s:s + CL, :])
        in_maps.append({"xT": xT, "wqkv": w, "bqkv": bq, "wproj": wp})
    return in_maps


def gather_outputs(results, b_proj):
    b_proj = np.asarray(b_proj, np.float32)
    y = np.zeros((B * T, C), np.float64)
    for r in results:
        y += r["y_part"]
    y = (y + b_proj).astype(np.float32).reshape(B, T, C)
    k = np.concatenate(
        [r["k_out"].reshape(B, T, HL, DK) for r in results], axis=2)
    v = np.concatenate(
        [r["v_out"].reshape(B, T, HL, DK) for r in results], axis=2)
    k = np.ascontiguousarray(k.transpose(0, 2, 1, 3))
    v = np.ascontiguousarray(v.transpose(0, 2, 1, 3))
    return y, k, v


_NC = None


def _get_nc():
    global _NC
    if _NC is None:
        _NC = build_program()
    return _NC


def kernel(x, W_qkv, b_qkv, W_proj, b_proj, **run_kwargs):
    nc = _get_nc()
    in_maps = shard_inputs(x, W_qkv, b_qkv, W_proj)
    res = run_bass_kernel_spmd(nc, in_maps, core_ids=list(range(N_CORES)),
                               **run_kwargs)
    y, k, v = gather_outputs(res.results, b_proj)
    kernel.last_results = res
    return (y, (k, v))


# revision 43
# speedup vs baseline: 1.1608x; 1.0041x over previous
"""Causal self-attention with KV cache — Trainium2 Bass kernel.

Strategy: tensor-parallel over heads. 16 heads / 8 cores = 2 heads per core.
Each core computes qkv projection for its 2 heads, causal attention, and a
partial output projection (its 128 columns of the c_proj contraction). The
host sums the 8 partial projections and assembles the k/v cache outputs.

Device layout is "transposed": qT/kT/vT live as [head*dk, tokens] so that
 - qkv projection runs with W stationary / xT moving (full PE efficiency),
 - score blocks come out as PT = scores.T [k, q] (softmax denom via a
   ones-column in the AV stationary operand),
 - attention output yT [c_local, tokens] is directly the lhsT the c_proj
   matmul needs (K=128, full efficiency).
All matmuls run as float32r (full rate at moving dim >= 256).
"""

import numpy as np

import concourse.bass as bass
import concourse.mybir as mybir
import concourse.tile as tile
from concourse import bacc
from concourse.bass_utils import run_bass_kernel_spmd
from concourse.masks import make_identity

F32 = mybir.dt.float32
F32R = mybir.dt.float32r

B = 2
T = 2048
C = 1024
N_HEADS = 16
DK = 64
N_CORES = 8
HL = N_HEADS // N_CORES      # heads per core = 2
CL = HL * DK                 # local channels = 128
QKVW = 3 * CL                # local qkv width = 384
KCH = C // 128               # emb contraction chunks = 8
QTILE = 512                  # q tile (moving dim)
NQT = T // QTILE             # q tiles per (b,h) = 4
NKC = T // 128               # k chunks per batch = 16
SCALE = 0.125                # 1/sqrt(DK)


def _r(ap):
    return ap.bitcast(F32R)


def _emit(nc, tc, io):
    xT_d, wqkv_d, bqkv_d, wproj_d, ypart_d, kout_d, vout_d = io

    pools = {}

    def pool(name, bufs, space="SBUF"):
        if name not in pools:
            pools[name] = tc.alloc_tile_pool(name=name, bufs=bufs, space=space)
        return pools[name]

    const_p = pool("const", 1)
    x_p = pool("x", 2)
    qkvT_p = pool("qkvT", 2)
    vn_p = pool("vn", 2)
    kn_p = pool("kn", 2)
    pt_p = pool("pt", 4)
    yt_p = pool("yt", 3)
    rec_p = pool("rec", 2)
    yo_p = pool("yo", 2)
    mm_ps = pool("mm_ps", 2, space="PSUM")
    pt_ps = pool("pt_ps", 3, space="PSUM")
    acc_ps = pool("acc_ps", 3, space="PSUM")

    # ---- constants ----
    w_sb = const_p.tile([128, KCH, QKVW], F32R, name="w_sb")
    for k in range(KCH):
        nc.sync.dma_start(w_sb[:, k, :], wqkv_d[k * 128:(k + 1) * 128, :])
    wp_sb = const_p.tile([128, C], F32R, name="wp_sb")
    b_sb = const_p.tile([128, 3], F32, name="b_sb")
    nc.sync.dma_start(b_sb[:], bqkv_d.rearrange("m p -> p m"))
    identity_f32 = const_p.tile([128, 128], F32, name="identity_f32")
    make_identity(nc, identity_f32)
    identity = const_p.tile([128, 128], F32R, name="identity")
    nc.vector.tensor_copy(identity[:], identity_f32[:])
    ones_f32 = const_p.tile([128, NKC * HL], F32, name="ones_f32")
    nc.gpsimd.memset(ones_f32[:], 1.0)
    # sel_h[0, m] = 1.0 iff m in head h's channel block (recip broadcast)
    sel = []
    for h in range(HL):
        sf = const_p.tile([1, 128], F32, name=f"self{h}")
        nc.gpsimd.memset(sf[:], 0.0)
        nc.gpsimd.memset(sf[:, h * DK:(h + 1) * DK], 1.0)
        sh = const_p.tile([1, 128], F32R, name=f"sel{h}")
        nc.vector.tensor_copy(sh[:], sf[:])
        sel.append(sh)

    def emit_qkv(b, j, dests):
        qT, kT, vT = dests
        tt = b * NQT + j
        x_sb = x_p.tile([128, KCH, QTILE], F32R, name="x_sb")
        for k in range(KCH):
            nc.sync.dma_start(
                x_sb[:, k, :],
                xT_d[k * 128:(k + 1) * 128, tt * QTILE:(tt + 1) * QTILE])
        for m in range(3):
            ps = mm_ps.tile([128, QTILE], F32, name="qkv_ps", tag="mm")
            for k in range(KCH):
                nc.tensor.matmul(
                    ps[:], w_sb[:, k, m * 128:(m + 1) * 128], x_sb[:, k, :],
                    start=(k == 0), stop=(k == KCH - 1))
            nc.vector.tensor_scalar_add(
                (qT, kT, vT)[m][:, j * QTILE:(j + 1) * QTILE], ps[:],
                b_sb[:, m:m + 1])

    def emit_transposes(b, dests, vn, kn):
        qT, kT, vT = dests
        nc.vector.tensor_copy(
            vn[:, :, :, DK:DK + 1],
            ones_f32.rearrange("p (a b c) -> p a b c", b=HL, c=1))
        for ci in range(NKC):
            trp = acc_ps.tile([128, 128], F32R, name="trp", tag="acc")
            nc.tensor.transpose(
                trp[:], vT[:, ci * 128:(ci + 1) * 128], identity[:])
            nc.vector.tensor_copy(
                vn[:, ci, :, 0:DK], trp.rearrange("p (h d) -> p h d", h=HL))
            trk = acc_ps.tile([128, 128], F32R, name="trk", tag="acc")
            nc.tensor.transpose(
                trk[:], kT[:, ci * 128:(ci + 1) * 128], identity[:])
            nc.vector.tensor_copy(kn[:, ci, :], trk[:])

    def emit_attention(b, qb, dests, vn):
        """One q-tile: both heads' chains interleaved pair-by-pair."""
        qT, kT, vT = dests
        npair = 2 * qb + 2             # k-chunk pairs (chunks 0..4qb+3)
        rec = [rec_p.tile([1, QTILE], F32R, name=f"rec{h}", tag=f"rec{h}")
               for h in range(HL)]
        yt = yt_p.tile([128, QTILE], F32R, name="yt")
        y_ps = [acc_ps.tile([DK + 1, QTILE], F32, name=f"y_ps{h}", tag="acc")
                for h in range(HL)]
        def do_scores(kc):
            w0 = max(0, 128 * (kc - 4 * qb))
            out = []
            for h in range(HL):
                kTh = kT[h * DK:(h + 1) * DK, :]
                qTh = qT[h * DK:(h + 1) * DK, qb * QTILE:(qb + 1) * QTILE]
                p_ps = pt_ps.tile([128, QTILE], F32, name="p_ps", tag="pt")
                nc.tensor.matmul(
                    p_ps[:, w0:], kTh[:, kc * 128:(kc + 1) * 128],
                    qTh[:, w0:], start=True, stop=True)
                p_sb = pt_p.tile([128, QTILE], F32R, name=f"p_sb{h}",
                                 tag=f"p_sb{h}")
                nc.scalar.activation(
                    p_sb[:, w0:], p_ps[:, w0:],
                    mybir.ActivationFunctionType.Exp, scale=SCALE)
                out.append(p_sb)
            return out

        def do_avs(kc, p_sbs):
            r = kc - 4 * qb
            w0 = max(0, 128 * r)
            for h in range(HL):
                if r >= 0:
                    # keep where kp <= qf - 128*r (triangle at window start)
                    nc.gpsimd.affine_select(
                        out=p_sbs[h][:, w0:], in_=p_sbs[h][:, w0:],
                        compare_op=mybir.AluOpType.is_ge, fill=0.0,
                        base=0, channel_multiplier=-1,
                        pattern=[[1, QTILE - w0]])
                nc.tensor.matmul(
                    y_ps[h][:, w0:], vn[:, kc, h, :], p_sbs[h][:, w0:],
                    start=(kc == 0), stop=(kc == 4 * qb + 3))

        nkc = 4 * qb + 4
        pend = []
        for kc in range(nkc):
            pend.append((kc, do_scores(kc)))
            if len(pend) > 2:
                k0, p0 = pend.pop(0)
                do_avs(k0, p0)
        for k0, p0 in pend:
            do_avs(k0, p0)
        for h in range(HL):
            with nc.allow_low_precision(reason="fp32r softmax recip"):
                nc.vector.reciprocal(rec[h][:], y_ps[h][DK:DK + 1, :])
            nc.vector.tensor_copy(yt[h * DK:(h + 1) * DK, :], y_ps[h][0:DK, :])
        # broadcast per-head reciprocals across partitions: [1,q] -> [128,q]
        br_ps = mm_ps.tile([128, QTILE], F32, name="br_ps", tag="mm")
        for h in range(HL):
            nc.tensor.matmul(br_ps[:], sel[h][:], rec[h][:],
                             start=(h == 0), stop=(h == HL - 1))
        nc.vector.tensor_mul(yt[:], yt[:], br_ps[:])
        # partial projection: y_part[tok, :] += yt.T @ wp  (K = 128)
        for ts in range(4):
            r0 = b * T + qb * QTILE + ts * 128
            yo = yo_p.tile([128, 2, 512], mybir.dt.bfloat16, name="yo")
            for nh in range(2):
                pp = mm_ps.tile([128, 512], F32, name="pp", tag="mm")
                nc.tensor.matmul(
                    pp[:], yt[:, ts * 128:(ts + 1) * 128],
                    wp_sb[:, nh * 512:(nh + 1) * 512],
                    start=True, stop=True)
                nc.vector.tensor_copy(yo[:, nh, :], pp[:])
            nc.sync.dma_start(
                ypart_d[r0:r0 + 128, :], yo.rearrange("p a b -> p (a b)"))

    def emit_kv_out(b, vn, kn):
        for h in range(HL):
            nc.sync.dma_start(
                vout_d[b].rearrange("(c p) (h d) -> p c h d", p=128, h=HL)[
                    :, :, h, :],
                vn[:, :, h, 0:DK].bitcast(F32))
        nc.sync.dma_start(
            kout_d[b].rearrange("(c p) n -> p c n", p=128), kn[:])

    dests, vns, kns = [], [], []
    for b in range(B):
        qT = qkvT_p.tile([128, T], F32R, name=f"qT{b}", tag="qT")
        kT = qkvT_p.tile([128, T], F32R, name=f"kT{b}", tag="kT")
        vT = qkvT_p.tile([128, T], F32R, name=f"vT{b}", tag="vT")
        dests.append((qT, kT, vT))
        vns.append(vn_p.tile([128, NKC, HL, DK + 1], F32R, name=f"vn{b}"))
        kns.append(kn_p.tile([128, NKC, 128], F32, name=f"kn{b}"))

    # batch 0 projections + transposes
    for j in range(NQT):
        emit_qkv(0, j, dests[0])
        if j == 0:
            nc.sync.dma_start(wp_sb[:], wproj_d[:, :])
    emit_transposes(0, dests[0], vns[0], kns[0])
    # batch 0 attention, with batch 1 qkv tiles emitted between q-tiles so
    # the PE has independent work whenever an attention chain waits on exp
    for qb in range(NQT):
        emit_attention(0, qb, dests[0], vns[0])
        emit_qkv(1, qb, dests[1])
    emit_kv_out(0, vns[0], kns[0])
    emit_transposes(1, dests[1], vns[1], kns[1])
    for qb in range(NQT):
        emit_attention(1, qb, dests[1], vns[1])
    emit_kv_out(1, vns[1], kns[1])

    for p in reversed(list(pools.values())):
        p.release()


def build_program():
    nc = bacc.Bacc("TRN2", target_bir_lowering=False, debug=False,
                   enable_asserts=False, num_devices=N_CORES)
    xT_d = nc.dram_tensor("xT", [C, B * T], F32R, kind="ExternalInput").ap()
    wqkv_d = nc.dram_tensor("wqkv", [C, QKVW], F32R, kind="ExternalInput").ap()
    bqkv_d = nc.dram_tensor("bqkv", [3, CL], F32, kind="ExternalInput").ap()
    wproj_d = nc.dram_tensor("wproj", [CL, C], F32R, kind="ExternalInput").ap()
    ypart_d = nc.dram_tensor("y_part", [B * T, C], mybir.dt.bfloat16, kind="ExternalOutput").ap()
    kout_d = nc.dram_tensor("k_out", [B, T, CL], F32, kind="ExternalOutput").ap()
    vout_d = nc.dram_tensor("v_out", [B, T, CL], F32, kind="ExternalOutput").ap()
    io = (xT_d, wqkv_d, bqkv_d, wproj_d, ypart_d, kout_d, vout_d)
    with tile.TileContext(nc) as tc:
        _emit(nc, tc, io)
    nc.compile()
    return nc


def shard_inputs(x, W_qkv, b_qkv, W_proj):
    x = np.asarray(x, np.float32)
    W_qkv = np.asarray(W_qkv, np.float32)
    b_qkv = np.asarray(b_qkv, np.float32)
    W_proj = np.asarray(W_proj, np.float32)
    xT = np.ascontiguousarray(x.reshape(B * T, C).T)
    in_maps = []
    for c in range(N_CORES):
        s = c * CL
        w = np.ascontiguousarray(np.concatenate(
            [W_qkv[:, s:s + CL], W_qkv[:, C + s:C + s + CL],
             W_qkv[:, 2 * C + s:2 * C + s + CL]], axis=1))
        bq = np.ascontiguousarray(np.stack(
            [b_qkv[s:s + CL], b_qkv[C + s:C + s + CL],
             b_qkv[2 * C + s:2 * C + s + CL]]).reshape(3, CL))
        wp = np.ascontiguousarray(W_proj[s:s + CL, :])
        in_maps.append({"xT": xT, "wqkv": w, "bqkv": bq, "wproj": wp})
    return in_maps


def gather_outputs(results, b_proj):
    b_proj = np.asarray(b_proj, np.float32)
    y = np.zeros((B * T, C), np.float64)
    for r in results:
        y += r["y_part"]
    y = (y + b_proj).astype(np.float32).reshape(B, T, C)
    k = np.concatenate(
        [r["k_out"].reshape(B, T, HL, DK) for r in results], axis=2)
    v = np.concatenate(
        [r["v_out"].reshape(B, T, HL, DK) for r in results], axis=2)
    k = np.ascontiguousarray(k.transpose(0, 2, 1, 3))
    v = np.ascontiguousarray(v.transpose(0, 2, 1, 3))
    return y, k, v


_NC = None


def _get_nc():
    global _NC
    if _NC is None:
        _NC = build_program()
    return _NC


def kernel(x, W_qkv, b_qkv, W_proj, b_proj, **run_kwargs):
    nc = _get_nc()
    in_maps = shard_inputs(x, W_qkv, b_qkv, W_proj)
    res = run_bass_kernel_spmd(nc, in_maps, core_ids=list(range(N_CORES)),
                               **run_kwargs)
    y, k, v = gather_outputs(res.results, b_proj)
    kernel.last_results = res
    return (y, (k, v))


# revision 44
# speedup vs baseline: 1.1953x; 1.0297x over previous
"""Causal self-attention with KV cache — Trainium2 Bass kernel.

Strategy: tensor-parallel over heads. 16 heads / 8 cores = 2 heads per core.
Each core computes qkv projection for its 2 heads, causal attention, and a
partial output projection (its 128 columns of the c_proj contraction). The
host sums the 8 partial projections and assembles the k/v cache outputs.

Device layout is "transposed": qT/kT/vT live as [head*dk, tokens] so that
 - qkv projection runs with W stationary / xT moving (full PE efficiency),
 - score blocks come out as PT = scores.T [k, q] (softmax denom via a
   ones-column in the AV stationary operand),
 - attention output yT [c_local, tokens] is directly the lhsT the c_proj
   matmul needs (K=128, full efficiency).
All matmuls run as float32r (full rate at moving dim >= 256).
"""

import numpy as np

import concourse.bass as bass
import concourse.mybir as mybir
import concourse.tile as tile
from concourse import bacc
from concourse.bass_utils import run_bass_kernel_spmd
from concourse.masks import make_identity

F32 = mybir.dt.float32
F32R = mybir.dt.float32r

B = 2
T = 2048
C = 1024
N_HEADS = 16
DK = 64
N_CORES = 8
HL = N_HEADS // N_CORES      # heads per core = 2
CL = HL * DK                 # local channels = 128
QKVW = 3 * CL                # local qkv width = 384
KCH = C // 128               # emb contraction chunks = 8
QTILE = 512                  # q tile (moving dim)
NQT = T // QTILE             # q tiles per (b,h) = 4
NKC = T // 128               # k chunks per batch = 16
SCALE = 0.125                # 1/sqrt(DK)


def _r(ap):
    return ap.bitcast(F32R)


def _emit(nc, tc, io):
    xT_d, wqkv_d, bqkv_d, wproj_d, ypart_d, kout_d, vout_d = io

    pools = {}

    def pool(name, bufs, space="SBUF"):
        if name not in pools:
            pools[name] = tc.alloc_tile_pool(name=name, bufs=bufs, space=space)
        return pools[name]

    const_p = pool("const", 1)
    x_p = pool("x", 2)
    qkvT_p = pool("qkvT", 2)
    vn_p = pool("vn", 2)
    kn_p = pool("kn", 2)
    pt_p = pool("pt", 4)
    yt_p = pool("yt", 3)
    rec_p = pool("rec", 2)
    yo_p = pool("yo", 2)
    mm_ps = pool("mm_ps", 2, space="PSUM")
    pt_ps = pool("pt_ps", 3, space="PSUM")
    acc_ps = pool("acc_ps", 3, space="PSUM")

    # ---- constants ----
    w_sb = const_p.tile([128, KCH, QKVW], F32R, name="w_sb")
    for k in range(KCH):
        nc.sync.dma_start(w_sb[:, k, :], wqkv_d[k * 128:(k + 1) * 128, :])
    wp_sb = const_p.tile([128, C], F32R, name="wp_sb")
    b_sb = const_p.tile([128, 3], F32, name="b_sb")
    nc.sync.dma_start(b_sb[:], bqkv_d.rearrange("m p -> p m"))
    identity_f32 = const_p.tile([128, 128], F32, name="identity_f32")
    make_identity(nc, identity_f32)
    identity = const_p.tile([128, 128], F32R, name="identity")
    nc.vector.tensor_copy(identity[:], identity_f32[:])
    ones_f32 = const_p.tile([128, NKC * HL], F32, name="ones_f32")
    nc.gpsimd.memset(ones_f32[:], 1.0)
    # sel_h[0, m] = 1.0 iff m in head h's channel block (recip broadcast)
    sel = []
    for h in range(HL):
        sf = const_p.tile([1, 128], F32, name=f"self{h}")
        nc.gpsimd.memset(sf[:], 0.0)
        nc.gpsimd.memset(sf[:, h * DK:(h + 1) * DK], 1.0)
        sh = const_p.tile([1, 128], F32R, name=f"sel{h}")
        nc.vector.tensor_copy(sh[:], sf[:])
        sel.append(sh)

    def emit_qkv(b, j, dests):
        qT, kT, vT = dests
        tt = b * NQT + j
        x_sb = x_p.tile([128, KCH, QTILE], F32R, name="x_sb")
        for k in range(KCH):
            nc.sync.dma_start(
                x_sb[:, k, :],
                xT_d[k * 128:(k + 1) * 128, tt * QTILE:(tt + 1) * QTILE])
        for m in range(3):
            ps = mm_ps.tile([128, QTILE], F32, name="qkv_ps", tag="mm")
            for k in range(KCH):
                nc.tensor.matmul(
                    ps[:], w_sb[:, k, m * 128:(m + 1) * 128], x_sb[:, k, :],
                    start=(k == 0), stop=(k == KCH - 1))
            nc.vector.tensor_scalar_add(
                (qT, kT, vT)[m][:, j * QTILE:(j + 1) * QTILE], ps[:],
                b_sb[:, m:m + 1])

    def emit_transposes(b, dests, vn, kn):
        qT, kT, vT = dests
        nc.vector.tensor_copy(
            vn[:, :, :, DK:DK + 1],
            ones_f32.rearrange("p (a b c) -> p a b c", b=HL, c=1))
        for ci in range(NKC):
            trp = acc_ps.tile([128, 128], F32R, name="trp", tag="acc")
            nc.tensor.transpose(
                trp[:], vT[:, ci * 128:(ci + 1) * 128], identity[:])
            nc.vector.tensor_copy(
                vn[:, ci, :, 0:DK], trp.rearrange("p (h d) -> p h d", h=HL))
            trk = acc_ps.tile([128, 128], F32R, name="trk", tag="acc")
            nc.tensor.transpose(
                trk[:], kT[:, ci * 128:(ci + 1) * 128], identity[:])
            nc.vector.tensor_copy(kn[:, ci, :], trk[:])

    def emit_attention(b, qb, dests, vn):
        """One q-tile: both heads' chains interleaved pair-by-pair."""
        qT, kT, vT = dests
        npair = 2 * qb + 2             # k-chunk pairs (chunks 0..4qb+3)
        rec = [rec_p.tile([1, QTILE], F32R, name=f"rec{h}", tag=f"rec{h}")
               for h in range(HL)]
        yt = yt_p.tile([128, QTILE], F32R, name="yt")
        y_ps = [acc_ps.tile([DK + 1, QTILE], F32, name=f"y_ps{h}", tag="acc")
                for h in range(HL)]
        def do_scores(kc):
            w0 = max(0, 128 * (kc - 4 * qb))
            out = []
            for h in range(HL):
                kTh = kT[h * DK:(h + 1) * DK, :]
                qTh = qT[h * DK:(h + 1) * DK, qb * QTILE:(qb + 1) * QTILE]
                p_ps = pt_ps.tile([128, QTILE], F32, name="p_ps", tag="pt")
                nc.tensor.matmul(
                    p_ps[:, w0:], kTh[:, kc * 128:(kc + 1) * 128],
                    qTh[:, w0:], start=True, stop=True)
                p_sb = pt_p.tile([128, QTILE], F32R, name=f"p_sb{h}",
                                 tag=f"p_sb{h}")
                nc.scalar.activation(
                    p_sb[:, w0:], p_ps[:, w0:],
                    mybir.ActivationFunctionType.Exp, scale=SCALE)
                out.append(p_sb)
            return out

        def do_avs(kc, p_sbs):
            r = kc - 4 * qb
            w0 = max(0, 128 * r)
            for h in range(HL):
                if r >= 0:
                    # keep where kp <= qf - 128*r (triangle at window start)
                    nc.gpsimd.affine_select(
                        out=p_sbs[h][:, w0:], in_=p_sbs[h][:, w0:],
                        compare_op=mybir.AluOpType.is_ge, fill=0.0,
                        base=0, channel_multiplier=-1,
                        pattern=[[1, QTILE - w0]])
                nc.tensor.matmul(
                    y_ps[h][:, w0:], vn[:, kc, h, :], p_sbs[h][:, w0:],
                    start=(kc == 0), stop=(kc == 4 * qb + 3))

        nkc = 4 * qb + 4
        pend = []
        for kc in range(nkc):
            pend.append((kc, do_scores(kc)))
            if len(pend) > 2:
                k0, p0 = pend.pop(0)
                do_avs(k0, p0)
        for k0, p0 in pend:
            do_avs(k0, p0)
        for h in range(HL):
            with nc.allow_low_precision(reason="fp32r softmax recip"):
                nc.vector.reciprocal(rec[h][:], y_ps[h][DK:DK + 1, :])
            nc.vector.tensor_copy(yt[h * DK:(h + 1) * DK, :], y_ps[h][0:DK, :])
        # broadcast per-head reciprocals across partitions: [1,q] -> [128,q]
        br_ps = mm_ps.tile([128, QTILE], F32, name="br_ps", tag="mm")
        for h in range(HL):
            nc.tensor.matmul(br_ps[:], sel[h][:], rec[h][:],
                             start=(h == 0), stop=(h == HL - 1))
        nc.vector.tensor_mul(yt[:], yt[:], br_ps[:])
        # partial projection: y_part[tok, :] += yt.T @ wp  (K = 128)
        for ts in range(4):
            r0 = b * T + qb * QTILE + ts * 128
            yo = yo_p.tile([128, 2, 512], F32, name="yo")
            for nh in range(2):
                pp = mm_ps.tile([128, 512], F32, name="pp", tag="mm")
                nc.tensor.matmul(
                    pp[:], yt[:, ts * 128:(ts + 1) * 128],
                    wp_sb[:, nh * 512:(nh + 1) * 512],
                    start=True, stop=True)
                nc.vector.tensor_copy(yo[:, nh, :], pp[:])
            nc.sync.dma_start(
                ypart_d[r0:r0 + 128, :], yo.rearrange("p a b -> p (a b)"))

    def emit_kv_out(b, vn, kn):
        for h in range(HL):
            nc.sync.dma_start(
                vout_d[b].rearrange("(c p) (h d) -> p c h d", p=128, h=HL)[
                    :, :, h, :],
                vn[:, :, h, 0:DK].bitcast(F32))
        nc.sync.dma_start(
            kout_d[b].rearrange("(c p) n -> p c n", p=128), kn[:])

    dests, vns, kns = [], [], []
    for b in range(B):
        qT = qkvT_p.tile([128, T], F32R, name=f"qT{b}", tag="qT")
        kT = qkvT_p.tile([128, T], F32R, name=f"kT{b}", tag="kT")
        vT = qkvT_p.tile([128, T], F32R, name=f"vT{b}", tag="vT")
        dests.append((qT, kT, vT))
        vns.append(vn_p.tile([128, NKC, HL, DK + 1], F32R, name=f"vn{b}"))
        kns.append(kn_p.tile([128, NKC, 128], F32, name=f"kn{b}"))

    # batch 0 projections + transposes
    for j in range(NQT):
        emit_qkv(0, j, dests[0])
        if j == 0:
            nc.sync.dma_start(wp_sb[:], wproj_d[:, :])
    emit_transposes(0, dests[0], vns[0], kns[0])
    # batch 0 attention, with batch 1 qkv tiles emitted between q-tiles so
    # the PE has independent work whenever an attention chain waits on exp
    for qb in range(NQT):
        emit_attention(0, qb, dests[0], vns[0])
        emit_qkv(1, qb, dests[1])
    emit_kv_out(0, vns[0], kns[0])
    emit_transposes(1, dests[1], vns[1], kns[1])
    for qb in range(NQT):
        emit_attention(1, qb, dests[1], vns[1])
    emit_kv_out(1, vns[1], kns[1])

    for p in reversed(list(pools.values())):
        p.release()


def build_program():
    nc = bacc.Bacc("TRN2", target_bir_lowering=False, debug=False,
                   enable_asserts=False, num_devices=N_CORES)
    xT_d = nc.dram_tensor("xT", [C, B * T], F32R, kind="ExternalInput").ap()
    wqkv_d = nc.dram_tensor("wqkv", [C, QKVW], F32R, kind="ExternalInput").ap()
    bqkv_d = nc.dram_tensor("bqkv", [3, CL], F32, kind="ExternalInput").ap()
    wproj_d = nc.dram_tensor("wproj", [CL, C], F32R, kind="ExternalInput").ap()
    ypart_d = nc.dram_tensor("y_part", [B * T, C], F32, kind="ExternalOutput").ap()
    kout_d = nc.dram_tensor("k_out", [B, T, CL], F32, kind="ExternalOutput").ap()
    vout_d = nc.dram_tensor("v_out", [B, T, CL], F32, kind="ExternalOutput").ap()
    io = (xT_d, wqkv_d, bqkv_d, wproj_d, ypart_d, kout_d, vout_d)
    with tile.TileContext(nc) as tc:
        _emit(nc, tc, io)
    nc.compile()
    return nc


def shard_inputs(x, W_qkv, b_qkv, W_proj):
    x = np.asarray(x, np.float32)
    W_qkv = np.asarray(W_qkv, np.float32)
    b_qkv = np.asarray(b_qkv, np.float32)
    W_proj = np.asarray(W_proj, np.float32)
    xT = np.ascontiguousarray(x.reshape(B * T, C).T)
    in_maps = []
    for c in range(N_CORES):
        s = c * CL
        w = np.ascontiguousarray(np.concatenate(
            [W_qkv[:, s:s + CL], W_qkv[:, C + s:C + s + CL],
             W_qkv[:, 2 * C + s:2 * C + s + CL]], axis=1))
        bq = np.ascontiguousarray(np.stack(
            [b_qkv[s:s + CL], b_qkv[C + s:C + s + CL],
             b_qkv[2 * C + s:2 * C + s + CL]]).reshape(3, CL))
        wp = np.ascontiguousarray(W_proj[s:s + CL, :])
        in_maps.append({"xT": xT, "wqkv": w, "bqkv": bq, "wproj": wp})
    return in_maps


def gather_outputs(results, b_proj):
    b_proj = np.asarray(b_proj, np.float32)
    y = np.zeros((B * T, C), np.float64)
    for r in results:
        y += r["y_part"]
    y = (y + b_proj).astype(np.float32).reshape(B, T, C)
    k = np.concatenate(
        [r["k_out"].reshape(B, T, HL, DK) for r in results], axis=2)
    v = np.concatenate(
        [r["v_out"].reshape(B, T, HL, DK) for r in results], axis=2)
    k = np.ascontiguousarray(k.transpose(0, 2, 1, 3))
    v = np.ascontiguousarray(v.transpose(0, 2, 1, 3))
    return y, k, v


_NC = None


def _get_nc():
    global _NC
    if _NC is None:
        _NC = build_program()
    return _NC


def kernel(x, W_qkv, b_qkv, W_proj, b_proj, **run_kwargs):
    nc = _get_nc()
    in_maps = shard_inputs(x, W_qkv, b_qkv, W_proj)
    res = run_bass_kernel_spmd(nc, in_maps, core_ids=list(range(N_CORES)),
                               **run_kwargs)
    y, k, v = gather_outputs(res.results, b_proj)
    kernel.last_results = res
    return (y, (k, v))
